# revision 31
# baseline (speedup 1.0000x reference)
"""DFlashAttention Trainium2 kernel (8 NeuronCores, SPMD, no collectives).

Problem (hardcoded shapes): B=4, QL=1024, CL=3072, KL=4096, H=2048,
NH=16 q-heads, NKV=4 kv-heads, HD=128.

Sharding: core i = (batch b = i//2, head-group g = i%2). Each core computes
8 q-heads / 2 kv-heads for one batch and produces a partial o_proj output
(contraction over its head block of Wo); the host sums the two partials per
batch (the "all-reduce after o_proj", done on host).

Design (lineage: 1104us -> ... -> 587us -> this round):
  - Host pre-transposes kv/cos/sin to h-major and lays weights out
    partition-major (contiguous DMA runs); everything bf16, fp32 PSUM.
  - rmsnorm: Square on ACT, partition-sum via ones-matmul, fast reciprocal,
    w folded into the rstd broadcast outer-product.
  - Chunk order [6,7,0..5]: Q (noise chunks) is projected FIRST so the ATT
    stage never waits on the Q norm chain, and the last K chunk's norm tail
    is covered by the ATT pipeline fill.
  - Attention: S^T pair into a 2-bank PSUM tile -> ONE [128,1024] exp
    (bf16); softmax denominators via a per-head DVE binary add-tree over
    the exp tiles + ONE tiny [1,512] ones-matmul per (head,qc) -- dens is
    ready per-head, so 1/dens, its broadcast and the PV normalization are
    all emitted (deferred) inside the NEXT head's S loop, off the critical
    path; o_proj starts right after head 7's chain.
  - Lag-2-pair software pipeline with alternating sT PSUM tags so PE never
    waits on a fresh exp; PV pairs share the V-tile stationary.
  - PSUM budget in ATT: sT(4 banks) + oT(2) + densP(1) + bc(1) = 8.
"""
import os
import sys

sys.path.insert(0, "/opt/trn_rl_repo")

import numpy as np
import ml_dtypes

import concourse.bass as bass
import concourse.tile as tile
from concourse import bacc, mybir
from concourse.bass_utils import run_bass_kernel_spmd

f32 = mybir.dt.float32
bf16 = mybir.dt.bfloat16
AF = mybir.ActivationFunctionType

P = 128
H = 2048
HT = H // P          # 16 h-tiles
QL = 1024
CL = 3072
KL = CL + QL         # 4096
KT_N = KL // P       # 32 k-tiles
HD = 128
NHC = 8              # q heads per core
NKVC = 2             # kv heads per core
SCALE = HD ** -0.5
EPS = 1e-6

_NC = None

CH_ORDER = [6, 7, 0, 1, 2, 3, 4, 5]


def build_nc():
    nc = bacc.Bacc("TRN2", target_bir_lowering=False, debug=False)

    kvT = nc.dram_tensor("kvt", [H, KL], bf16, kind="ExternalInput").ap()
    cosT = nc.dram_tensor("cost", [HD, KL], bf16, kind="ExternalInput").ap()
    sinT = nc.dram_tensor("sint", [HD, KL], bf16, kind="ExternalInput").ap()
    # weights pre-laid out partition-major on host: [P, HT*C] contiguous
    wq = nc.dram_tensor("wq", [P, HT * NHC * HD], bf16,
                        kind="ExternalInput").ap()
    wk = nc.dram_tensor("wk", [P, HT * NKVC * HD], bf16,
                        kind="ExternalInput").ap()
    wv = nc.dram_tensor("wv", [P, HT * NKVC * HD], bf16,
                        kind="ExternalInput").ap()
    wo = nc.dram_tensor("wo", [NHC * HD, H], bf16, kind="ExternalInput").ap()
    qnw = nc.dram_tensor("qnw", [1, HD], bf16, kind="ExternalInput").ap()
    knw = nc.dram_tensor("knw", [1, HD], bf16, kind="ExternalInput").ap()
    out = nc.dram_tensor("out", [QL, H], bf16, kind="ExternalOutput").ap()

    wq_r = wq.rearrange("p (ht c) -> p ht c", ht=HT)
    wk_r = wk.rearrange("p (ht c) -> p ht c", ht=HT)
    wv_r = wv.rearrange("p (ht c) -> p ht c", ht=HT)
    kvT_r = kvT.rearrange("(ht p) k -> p ht k", p=P)

    with tile.TileContext(nc) as tc:
        with tc.tile_pool(name="persist", bufs=1) as persist:
            # ---- persistent tiles (allocation only) ----
            rotm = persist.tile([P, P], bf16)
            ones_col = persist.tile([P, 1], bf16)
            # dens selectors: sel_c[:, r, :] = [128,16] with col r = 1
            sel_c = persist.tile([P, 16, 16], bf16)
            # broadcast selectors: selB[:, r, :] = [16,128], row r = 1
            selB = persist.tile([16, 16, P], bf16)
            qn_row = persist.tile([1, HD], bf16)
            kn_row = persist.tile([1, HD], bf16)
            eps_sb = persist.tile([1, 1], f32)
            QT = persist.tile([P, NHC, QL], bf16)    # Q'^T  [d, head, q]
            # K'^T / V split into PER-CHUNK tiles: dependency tracking is
            # tile-granular, so per-chunk tiles let the ATT stage's first
            # S/PV start without waiting for the last chunk's writes
            KT_parts = [persist.tile([P, NKVC, 512], bf16, name=f"KTp{i}")
                        for i in range(8)]           # [d, kvh, tok(chunk)]
            V_parts = [persist.tile([P, 4, NKVC * HD], bf16, name=f"Vp{i}")
                       for i in range(8)]            # [tok, tt, c]

            # ========= Stage QKV (Q first: chunks 6,7 then 0..5) =========
            with tc.tile_pool(name="kv_str", bufs=3) as kvp, \
                 tc.tile_pool(name="kv_w", bufs=1) as wp, \
                 tc.tile_pool(name="kv_mid", bufs=2) as midp, \
                 tc.tile_pool(name="kv_cst", bufs=2) as cstp, \
                 tc.tile_pool(name="kv_proj", bufs=3, space="PSUM") as projp, \
                 tc.tile_pool(name="kv_pv", bufs=2, space="PSUM") as pvp, \
                 tc.tile_pool(name="kv_ssq", bufs=1, space="PSUM") as ssqp, \
                 tc.tile_pool(name="kv_scl", bufs=1, space="PSUM") as sclp, \
                 tc.tile_pool(name="kv_rot", bufs=1, space="PSUM") as rotp:
                # --- issue the startup DMAs FIRST (before constants) ---
                # first chunk's tokens + wq arrive as 4 INDEPENDENT tiles
                # (deps are tile-granular: split tiles => early first matmul)
                ch0 = CH_ORDER[0]
                col0 = slice(ch0 * 512, (ch0 + 1) * 512)
                kv6 = [kvp.tile([P, 4, 512], bf16, name=f"kv6{i}", tag=f"kv6{i}", bufs=1)
                       for i in range(4)]
                wq_parts = [wp.tile([P, 4, NHC * HD], bf16, name=f"wqp{i}")
                            for i in range(4)]
                wk_sb = wp.tile([P, HT, NKVC * HD], bf16)
                wv_sb = wp.tile([P, HT, NKVC * HD], bf16)
                for i in range(4):
                    nc.sync.dma_start(out=kv6[i],
                                      in_=kvT_r[:, 4 * i:4 * i + 4, col0])
                    nc.sync.dma_start(out=wq_parts[i],
                                      in_=wq_r[:, 4 * i:4 * i + 4])
                cosT_c0 = cstp.tile([P, 512], bf16, tag="cosT")
                nc.sync.dma_start(out=cosT_c0, in_=cosT[:, col0])
                sinT_c0 = cstp.tile([P, 512], bf16, tag="sinT")
                nc.sync.dma_start(out=sinT_c0, in_=sinT[:, col0])
                nc.sync.dma_start(out=qn_row, in_=qnw)
                nc.sync.dma_start(out=kn_row, in_=knw)
                # prefetch the 2nd chunk before the K/V weights (Q for ch7
                # runs before ch6's K projection needs wk)
                ch1 = CH_ORDER[1]
                kvc7 = kvp.tile([P, HT, 512], bf16, tag="kvc7", bufs=1)
                nc.sync.dma_start(out=kvc7,
                                  in_=kvT_r[:, :, ch1 * 512:(ch1 + 1) * 512])
                nc.sync.dma_start(out=wk_sb[:, 0:8], in_=wk_r[:, 0:8])
                nc.sync.dma_start(out=wk_sb[:, 8:16], in_=wk_r[:, 8:16])
                nc.sync.dma_start(out=wv_sb[:, 0:8], in_=wv_r[:, 0:8])
                nc.sync.dma_start(out=wv_sb[:, 8:16], in_=wv_r[:, 8:16])

                # --- constants (gpsimd/DVE work overlaps the DMAs) ---
                with tc.tile_pool(name="cscratch", bufs=1) as csp:
                    rot_f = csp.tile([P, P], f32)
                    nc.gpsimd.memset(rot_f, 0.0)
                    # +1 where col = row + 64 (out[d'] = x[d'-64], d' >= 64)
                    nc.gpsimd.affine_select(
                        out=rot_f, in_=rot_f,
                        compare_op=mybir.AluOpType.not_equal,
                        fill=1.0, base=64, pattern=[[-1, P]],
                        channel_multiplier=1)
                    # -1 where col = row - 64 (out[d'] = -x[d'+64], d' < 64)
                    nc.gpsimd.affine_select(
                        out=rot_f, in_=rot_f,
                        compare_op=mybir.AluOpType.not_equal,
                        fill=-1.0, base=-64, pattern=[[-1, P]],
                        channel_multiplier=1)
                    nc.vector.tensor_copy(rotm, rot_f)
                with tc.tile_pool(name="selscr", bufs=1) as sscr:
                    selB_f = sscr.tile([16, 16, P], f32)
                    nc.gpsimd.memset(selB_f, 0.0)
                    nc.gpsimd.affine_select(
                        out=selB_f, in_=selB_f,
                        compare_op=mybir.AluOpType.not_equal,
                        fill=1.0, base=0, pattern=[[-1, 16], [0, P]],
                        channel_multiplier=1)
                    nc.vector.tensor_copy(selB, selB_f)
                nc.vector.memset(sel_c, 0.0)
                for r in range(16):
                    nc.vector.memset(sel_c[:, r, r:r + 1], 1.0)
                nc.vector.memset(ones_col, 1.0)
                nc.vector.memset(eps_sb, EPS)

                def norm_rope(ps, w_row, cosT_ap, sinT_ap, dst_ap, psums):
                    """ps [128,512] f32 PSUM -> dst_ap (bf16): rmsnorm+rope."""
                    ssqp_, sclp_, rotp_ = psums
                    sq = midp.tile([P, 512], bf16, tag="sq")
                    nc.scalar.activation(sq, ps, func=AF.Square)
                    ssq = ssqp_.tile([1, 512], f32, tag="ssq")
                    nc.tensor.matmul(ssq, ones_col, sq, start=True, stop=True)
                    srt = midp.tile([1, 512], f32, tag="srt", bufs=1)
                    nc.scalar.activation(srt, ssq, func=AF.Sqrt,
                                         scale=1.0 / HD, bias=eps_sb)
                    rstd = midp.tile([1, 512], f32, tag="rstd", bufs=1)
                    nc.vector.reciprocal_approx_fast(out=rstd, in_=srt)
                    rstd_b = midp.tile([1, 512], bf16, tag="rstd_b", bufs=1)
                    nc.vector.tensor_copy(rstd_b, rstd)
                    scl_ps = sclp_.tile([P, 512], f32, tag="scl_ps")
                    nc.tensor.matmul(scl_ps, w_row, rstd_b,
                                     start=True, stop=True)
                    scl = midp.tile([P, 512], f32, tag="scl", bufs=1)
                    nc.scalar.activation(scl, scl_ps, func=AF.Copy)
                    qn = midp.tile([P, 512], bf16, tag="qn")
                    nc.vector.tensor_mul(qn, ps, scl)
                    rot_ps = rotp_.tile([P, 512], f32, tag="rot_ps")
                    nc.tensor.matmul(rot_ps, rotm, qn, start=True, stop=True)
                    t1 = midp.tile([P, 512], bf16, tag="t1", bufs=1)
                    nc.vector.tensor_mul(t1, qn, cosT_ap)
                    t2 = midp.tile([P, 512], bf16, tag="t2", bufs=1)
                    nc.vector.tensor_mul(t2, rot_ps, sinT_ap)
                    nc.vector.tensor_add(dst_ap, t1, t2)

                for ci, ch in enumerate(CH_ORDER):
                    col = slice(ch * 512, (ch + 1) * 512)
                    if ci == 0:
                        kv_at = lambda ht, c=slice(0, 512): \
                            kv6[ht // 4][:, ht % 4, c]
                        cosT_c, sinT_c = cosT_c0, sinT_c0
                    else:
                        if ci == 1:
                            kvc = kvc7
                        else:
                            kvc = kvp.tile([P, HT, 512], bf16, tag="kvc")
                            nc.sync.dma_start(out=kvc, in_=kvT_r[:, :, col])
                        kv_at = lambda ht, c=slice(0, 512), t=kvc: \
                            t[:, ht, c]
                        cosT_c = cstp.tile([P, 512], bf16, tag="cosT")
                        nc.sync.dma_start(out=cosT_c, in_=cosT[:, col])
                        sinT_c = cstp.tile([P, 512], bf16, tag="sinT")
                        nc.sync.dma_start(out=sinT_c, in_=sinT[:, col])
                    # Q projection for the noise rows (chunks 6, 7)
                    if ch >= 6:
                        qc = ch - 6
                        for ct in range(NHC):
                            psq = projp.tile([P, 512], f32, tag="proj")
                            for ht in range(HT):
                                nc.tensor.matmul(
                                    psq,
                                    wq_parts[ht // 4][:, ht % 4,
                                                      ct * P:(ct + 1) * P],
                                    kv_at(ht),
                                    start=(ht == 0), stop=(ht == HT - 1))
                            norm_rope(psq, qn_row, cosT_c, sinT_c,
                                      QT[:, ct, qc * 512:(qc + 1) * 512],
                                      (ssqp, sclp, rotp))
                    # K^T projection + norm + rope (per kv head = 128 rows)
                    for ckt in range(NKVC):
                        ps = projp.tile([P, 512], f32, tag="proj")
                        for ht in range(HT):
                            nc.tensor.matmul(
                                ps, wk_sb[:, ht, ckt * HD:(ckt + 1) * HD],
                                kv_at(ht),
                                start=(ht == 0), stop=(ht == HT - 1))
                        norm_rope(ps, kn_row, cosT_c, sinT_c,
                                  KT_parts[ch][:, ckt, :],
                                  (ssqp, sclp, rotp))
                    # V projection (natural layout), resident in SBUF
                    for tt in range(4):
                        psv = pvp.tile([P, NKVC * HD], f32, tag="psv")
                        for ht in range(HT):
                            nc.tensor.matmul(
                                psv, kv_at(ht, slice(tt * P, (tt + 1) * P)),
                                wv_sb[:, ht, :],
                                start=(ht == 0), stop=(ht == HT - 1))
                        nc.vector.tensor_copy(V_parts[ch][:, tt, :], psv)

            # ========= Stage ATT (incl per-head normalize) + O =========
            with tc.tile_pool(name="post", bufs=1) as postp:
                OT = postp.tile([P, NHC, QL], bf16)     # normalized O^T
                wo_sb = postp.tile([P, NHC, H], bf16)
                nc.sync.dma_start(
                    out=wo_sb,
                    in_=wo.rearrange("(ci p) n -> p ci n", p=P))
                _stage_att(nc, tc, OT, KT_parts, QT, V_parts, sel_c, selB)
                _stage_o(nc, tc, OT, wo_sb, out)

    nc.compile()
    return nc


def _stage_att(nc, tc, OT, KT_parts, QT, V_parts, sel_c, selB):
    with tc.tile_pool(name="at_et", bufs=8) as etp, \
         tc.tile_pool(name="at_lv", bufs=2) as lvp, \
         tc.tile_pool(name="at_st", bufs=2, space="PSUM") as sTp, \
         tc.tile_pool(name="at_ot", bufs=1, space="PSUM") as oTp, \
         tc.tile_pool(name="at_den", bufs=1, space="PSUM") as denp, \
         tc.tile_pool(name="at_bc", bufs=1, space="PSUM") as bcp:
        carries = [None, None, None, None]    # dens binary add-tree
        tree_tot = [None]                     # completed per-head total
        rden_cur = [None]                     # per-head 1/dens [2,512] bf16

        def pv(peT_a, peT_b, poT, pr, ph):
            # PV for both kts of the pair (V-tile stationary shared per kt)
            kvh = ph // 4
            for peT, pkt in ((peT_a, 2 * pr), (peT_b, 2 * pr + 1)):
                for qc in range(2):
                    sl = slice(qc * 512, (qc + 1) * 512)
                    nc.tensor.matmul(
                        poT[:, sl],
                        V_parts[pkt // 4][:, pkt % 4,
                                          kvh * HD:(kvh + 1) * HD],
                        peT[:, sl],
                        start=(pkt == 0), stop=(pkt == KT_N - 1))

        def dens_head(lh):
            # dens rows (qc0, qc1) of a base-0 [2,512] PSUM tile from the
            # completed tree total, via 2-col selectors (matmul outputs and
            # custom-DVE ops must be partition-0 based). The reciprocal
            # chain runs on DVE while PE streams the next head's S matmuls.
            tot = tree_tot[0]
            tree_tot[0] = None
            densH = denp.tile([2, 512], f32, tag="dh")
            for qc in range(2):
                nc.tensor.matmul(densH, sel_c[:, qc, 0:2],
                                 tot[:, qc * 512:(qc + 1) * 512],
                                 start=(qc == 0), stop=(qc == 1))
            rdenH = lvp.tile([2, 512], f32, tag="rd")
            nc.vector.reciprocal_approx_fast(out=rdenH, in_=densH)
            rdenH_b = lvp.tile([2, 512], bf16, tag="rdb")
            nc.vector.tensor_copy(rdenH_b, rdenH)
            rden_cur[0] = rdenH_b

        def normalize_head(lh, oT, qc):
            # OT[:, lh, qc] = oT * broadcast(1/dens); bc reuses ONE psum
            # bank, so the two qc halves are emitted a pair apart (WAR)
            sl = slice(qc * 512, (qc + 1) * 512)
            bc = bcp.tile([P, 512], f32, tag="bc")
            nc.tensor.matmul(bc, selB[0:2, qc, :], rden_cur[0],
                             start=True, stop=True)
            # DVE may read only one PSUM operand: stage bc through SBUF
            bc_sb = lvp.tile([P, 512], f32, tag="bcs")
            nc.vector.tensor_copy(bc_sb, bc)
            nc.vector.tensor_mul(OT[:, lh, sl], oT[:, sl], bc_sb)

        prev = [None]  # (lh, oT) awaiting dens/normalize
        for lh in range(NHC):
            kvh = lh // 4
            oT = oTp.tile([P, QL], f32, tag="oT")
            pend = []  # lag-2-pair pipeline: PE never waits on a fresh exp
            for pr in range(KT_N // 2):
                # deferred prev-head chain; must fully precede this head's
                # first PV pop (pr==3) since oT is single-buffered
                if pr == 1 and prev[0] is not None:
                    dens_head(prev[0][0])
                if pr == 2 and prev[0] is not None:
                    normalize_head(prev[0][0], prev[0][1], 0)
                if pr == 3 and prev[0] is not None:
                    normalize_head(prev[0][0], prev[0][1], 1)
                    prev[0] = None
                eTs = []
                for kt in (2 * pr, 2 * pr + 1):
                    # alternate explicit tags so the S(kt) write and the
                    # exp(kt-1) read never touch the same PSUM slot object
                    sT = sTp.tile([P, QL], f32, tag=f"sT{kt % 2}", bufs=1)
                    for qc in range(2):
                        nc.tensor.matmul(
                            sT[:, qc * 512:(qc + 1) * 512],
                            KT_parts[kt // 4][:, kvh,
                                              (kt % 4) * P:(kt % 4 + 1) * P],
                            QT[:, lh, qc * 512:(qc + 1) * 512],
                            start=True, stop=True)
                    eT = etp.tile([P, QL], bf16, tag="eT")
                    nc.scalar.activation(eT, sT, func=AF.Exp, scale=SCALE)
                    eTs.append(eT)
                # dens add-tree climbs as exp tiles arrive (DVE slack);
                # bf16 tree depth 5 puts ~0.3% rms on dens, negligible
                node = lvp.tile([P, QL], bf16, tag="l0")
                nc.vector.tensor_add(node, eTs[0], eTs[1])
                lvl = 0
                while lvl < 4 and carries[lvl] is not None:
                    sib = carries[lvl]
                    carries[lvl] = None
                    nxt = lvp.tile([P, QL], bf16, tag=f"l{lvl + 1}")
                    nc.vector.tensor_add(nxt, sib, node)
                    node = nxt
                    lvl += 1
                if lvl == 4:
                    tree_tot[0] = node   # head total (16 pairs folded)
                else:
                    carries[lvl] = node
                pend.append((eTs[0], eTs[1], oT, pr, lh))
                if len(pend) > 3:
                    pv(*pend.pop(0))
            for args in pend:
                pv(*args)
            prev[0] = (lh, oT)
        # head 7: emit its chain immediately (short serial tail before O)
        dens_head(prev[0][0])
        normalize_head(prev[0][0], prev[0][1], 0)
        normalize_head(prev[0][0], prev[0][1], 1)


def _stage_o(nc, tc, OT, wo_sb, out):
    with tc.tile_pool(name="o_out", bufs=4) as outp, \
         tc.tile_pool(name="o_ps0", bufs=2, space="PSUM") as opsA, \
         tc.tile_pool(name="o_ps1", bufs=2, space="PSUM") as opsB:
        # o_proj: out[q, n] = sum_ci OT[:, ci, q].T @ wo[:, ci, n]
        for qt in range(8):
            for half in range(2):
                ps0 = opsA.tile([P, 512], f32, tag="ops0")
                ps1 = opsB.tile([P, 512], f32, tag="ops1")
                pss = (ps0, ps1)
                for ci in range(NHC):
                    for nch in range(2):
                        nc.tensor.matmul(
                            pss[nch], OT[:, ci, qt * P:(qt + 1) * P],
                            wo_sb[:, ci,
                                  half * 1024 + nch * 512:
                                  half * 1024 + (nch + 1) * 512],
                            start=(ci == 0), stop=(ci == NHC - 1))
                ob = outp.tile([P, 1024], bf16, tag="ob")
                nc.scalar.activation(ob[:, 0:512], ps0, func=AF.Copy)
                nc.vector.tensor_copy(ob[:, 512:1024], ps1)
                nc.sync.dma_start(
                    out=out[qt * P:(qt + 1) * P,
                            half * 1024:(half + 1) * 1024],
                    in_=ob)


def _get_nc():
    global _NC
    if _NC is None:
        _NC = build_nc()
    return _NC


def _wlayout(w):
    """[H, C] -> [P, HT*C] partition-major (contiguous per-partition DMA)."""
    Hh, C = w.shape
    return np.ascontiguousarray(
        w.reshape(HT, P, C).transpose(1, 0, 2).reshape(P, HT * C))


def _make_in_maps(noise, ctx, cos, sin, Wq, Wk, Wv, Wo, qn_w, kn_w):
    bf = ml_dtypes.bfloat16
    noise = np.asarray(noise, np.float32)
    ctx = np.asarray(ctx, np.float32)
    cos = np.asarray(cos, np.float32)
    sin = np.asarray(sin, np.float32)
    Wq = np.asarray(Wq, np.float32).astype(bf)
    Wk = np.asarray(Wk, np.float32).astype(bf)
    Wv = np.asarray(Wv, np.float32).astype(bf)
    Wo = np.asarray(Wo, np.float32).astype(bf)
    qn_w = np.asarray(qn_w, np.float32).reshape(1, HD).astype(bf)
    kn_w = np.asarray(kn_w, np.float32).reshape(1, HD).astype(bf)
    B = noise.shape[0]
    in_maps = []
    for b in range(B):
        kvT_b = np.ascontiguousarray(
            np.concatenate([ctx[b], noise[b]], axis=0).T).astype(bf)
        cosT_b = np.ascontiguousarray(cos[b].T).astype(bf)
        sinT_b = np.ascontiguousarray(sin[b].T).astype(bf)
        for g in range(2):
            in_maps.append({
                "kvt": kvT_b,
                "cost": cosT_b,
                "sint": sinT_b,
                "wq": _wlayout(Wq[:, g * 1024:(g + 1) * 1024]),
                "wk": _wlayout(Wk[:, g * 256:(g + 1) * 256]),
                "wv": _wlayout(Wv[:, g * 256:(g + 1) * 256]),
                "wo": np.ascontiguousarray(Wo[g * 1024:(g + 1) * 1024, :]),
                "qnw": qn_w,
                "knw": kn_w,
            })
    return in_maps


def _install_profile_hook():
    """Provide antenv.axon_hooks (absent in this container) so
    run_bass_kernel_spmd(trace=True) can NTFF-profile via libaxon_pjrt."""
    import types
    if "antenv.axon_hooks" not in sys.modules:
        import antenv
        mod = types.ModuleType("antenv.axon_hooks")
        _state = {}
        mod.set_axon_ntff_profile_hook = lambda h: _state.__setitem__("h", h)
        mod.get_axon_ntff_profile_hook = lambda: _state.get("h")
        sys.modules["antenv.axon_hooks"] = mod
        antenv.axon_hooks = mod
        from trn_agent_boot.trn_boot import _ntff_profile_via_ctypes
        mod.set_axon_ntff_profile_hook(
            _ntff_profile_via_ctypes("/opt/axon/libaxon_pjrt.so"))
    import concourse.bass_utils as bu
    bu.upload_artifacts = lambda tmpdir: tmpdir


def run(inputs, trace=False, tmpdir=None):
    """Run on 8 cores; returns (output [4,1024,2048], result)."""
    nc = _get_nc()
    in_maps = _make_in_maps(**inputs)
    if trace:
        _install_profile_hook()
    res = run_bass_kernel_spmd(nc, in_maps, core_ids=list(range(8)),
                               trace=trace, tmpdir=tmpdir,
                               trace_cores=[0] if trace else None)
    outs = [np.asarray(res.results[i]["out"], dtype=np.float32)
            for i in range(8)]
    full = np.stack([outs[2 * b] + outs[2 * b + 1] for b in range(4)], axis=0)
    return full.astype(np.float32), res


def kernel(**inputs):
    out, _ = run(inputs, trace=False)
    return out


def summarize_trace(res, top=30):
    """Per-engine busy time + top source lines by total duration."""
    if not res.instructions_and_trace:
        print("no trace")
        return
    insts, trace_path = res.instructions_and_trace
    from collections import defaultdict

    def _get(i, name):
        v = getattr(i, name, None)
        if callable(v):
            try:
                v = v()
            except Exception:
                v = "?"
        return v

    eng_busy = defaultdict(int)
    eng_n = defaultdict(int)
    line_cost = defaultdict(int)
    line_n = defaultdict(int)
    recs = []
    t0 = min(i.timestamp for i in insts)
    t1 = max(i.end_timestamp for i in insts)
    for i in insts:
        e = str(i.engine)
        op = str(_get(i, "op_name"))
        line = str(_get(i, "source_line"))
        eng_busy[e] += int(i.duration)
        eng_n[e] += 1
        line_cost[(e, op, line)] += int(i.duration)
        line_n[(e, op, line)] += 1
        recs.append((int(i.timestamp) - t0, int(i.end_timestamp) - t0,
                     int(i.duration), e, op, line))
    with open("/tmp/insts.tsv", "w") as f:
        for r in sorted(recs):
            f.write("\t".join(str(x) for x in r) + "\n")
    span = t1 - t0
    print(f"trace: {trace_path}")
    print(f"span: {span} ns   (insts dumped to /tmp/insts.tsv)")
    for e in sorted(eng_busy, key=lambda e: -eng_busy[e]):
        print(f"  {e:12s} busy {eng_busy[e]:>10} ns "
              f"({100.0 * eng_busy[e] / span:5.1f}%)  n={eng_n[e]}")
    print("top cost lines:")
    for (e, op, line), c in sorted(line_cost.items(),
                                   key=lambda kv: -kv[1])[:top]:
        print(f"  {c:>10} ns n={line_n[(e, op, line)]:>4}  {e:10s} "
              f"{op:26s} {line}")
